# Initial kernel scaffold
#
"""Expert-parallel MoE (top-2 of 8 experts, SwiGLU FFN) for 8 Trainium2 cores.

Strategy (matches the expert-parallel sharding hint):
  - Host computes the small gate (logits -> top-2 -> softmax) in float64
    numpy, then dispatches ("all-to-all" on host) tokens to experts.
  - Core e holds expert e's weights and runs a dense SwiGLU FFN over the
    tokens routed to expert e (padded to a common capacity C so all 8
    cores run the same SPMD program).
  - The device kernel works entirely in "feature-major" layout (features
    on partitions, tokens on the free axis) so the h = silu(x@Wg)*(x@Wu)
    intermediate feeds the down-projection without any transpose.
  - Host applies the routing weights and scatter-adds the per-expert
    outputs back into the full [B,T,D] output.

DTYPE selects the matmul path (both measured on HW):
  - "f32r" (default): fp32 bits on the relaxed-precision PE path,
    1 cycle/row + ~40 cycle/matmul weight-load bubble. Max-core HW time
    ~561 us, rel err vs f64 reference 2.6e-4.
  - "bf16": inputs rounded to bfloat16, 1 cycle/row, LDWEIGHTS hidden by
    fast-weight-load. ~508 us but rel err 4.1e-3 — kept as a fallback.
"""

import numpy as np

DIM = 1024
HID = 2816
E = 8
TOPK = 2
P = 128
KD = DIM // P   # 8 k-subtiles (contraction of x@W)
HT = HID // P   # 22 h-subtiles
DT = DIM // P   # 8 d-subtiles (output features)

DTYPE = "f32r"          # "bf16" | "f32r"
CHUNK_MAX = 512         # PSUM bank limit (512 fp32 accumulators)
CHUNK_MIN = 384         # keep weight-DMA per chunk sustainable

_KERNEL_CACHE = {}
LAST_RESULTS = None  # BassKernelResults of the most recent run (for profiling)


def _align():
    # f32r matmuls reject odd moving free dims (walrus ISA check);
    # bf16 takes any size.
    return 1 if DTYPE == "bf16" else 2


def _capacity(max_cnt):
    # chunks produced by _build_chunks are all >= CHUNK_MIN >= 256, so
    # both the bf16 and f32r matmul fast paths allow any capacity.
    a = _align()
    return max(CHUNK_MIN, ((max_cnt + a - 1) // a) * a)


def _build_chunks(C):
    """Split C into aligned chunks in [CHUNK_MIN, CHUNK_MAX], largest
    LAST: a trailing single-chunk weight group streams 1MB of wg/wu per
    h-tile, and only a full-width chunk keeps that under the ~360GB/s
    per-core HBM roofline."""
    count = (C + CHUNK_MAX - 1) // CHUNK_MAX
    sizes = []
    rem = C
    for i in range(count, 0, -1):
        if i == 1:
            s = rem
        else:
            s = min(CHUNK_MAX, rem - CHUNK_MIN * (i - 1))
        sizes.append(s)
        rem -= s
    sizes.reverse()
    assert all(CHUNK_MIN <= s <= CHUNK_MAX and s % _align() == 0 for s in sizes)
    chunks = []
    off = 0
    for s in sizes:
        chunks.append((off, s))
        off += s
    return chunks


def _build_groups(chunks, group_size):
    return [chunks[i : i + group_size] for i in range(0, len(chunks), group_size)]


def _build_moe_ffn(C):
    """Build the per-core Bass program: y^T = SwiGLU FFN of x^T, both
    feature-major, tokens padded to capacity C."""
    import concourse.bass as bass  # noqa: F401
    import concourse.mybir as mybir
    from concourse import bacc, tile

    f32 = mybir.dt.float32
    dt_in = mybir.dt.bfloat16 if DTYPE == "bf16" else mybir.dt.float32r
    SiLU = mybir.ActivationFunctionType.Silu

    nc = bacc.Bacc("TRN2", target_bir_lowering=False, debug=False)

    xt = nc.dram_tensor("xt", [P, KD, C], dt_in, kind="ExternalInput")
    wgt = nc.dram_tensor("wgt", [HT, P, KD, P], dt_in, kind="ExternalInput")
    wut = nc.dram_tensor("wut", [HT, P, KD, P], dt_in, kind="ExternalInput")
    wdt = nc.dram_tensor("wdt", [DT, P, HT, P], dt_in, kind="ExternalInput")
    yt = nc.dram_tensor("yt", [DT, P, C], f32, kind="ExternalOutput")

    # bf16 halves weight DMA, so single-chunk groups sustain; f32r needs
    # two chunks per weight pass to stay under the HBM roofline.
    group_size = 1 if DTYPE == "bf16" else 2
    groups = _build_groups(_build_chunks(C), group_size)

    with tile.TileContext(nc) as tc:
        with (
            tc.tile_pool(name="xp", bufs=1) as xp,
            tc.tile_pool(name="wp", bufs=3) as wp,
            tc.tile_pool(name="hp", bufs=2 if group_size == 1 else 1) as hp,
            tc.tile_pool(name="op", bufs=3) as op,
            tc.tile_pool(name="ps", bufs=2, space="PSUM") as ps,
        ):
            HH = KD // 2  # wg/wu tiles split in halves for earlier start
            for group in groups:
                g_off = group[0][0]
                g_size = sum(c[1] for c in group)

                # h = silu(x @ Wg) * (x @ Wu), feature-major [HID, g_size]
                h_sb = hp.tile([P, HT, g_size], dt_in, tag="h")

                # ht=0 weights first so the opening matmuls wait on
                # ~0.75MB, not the whole group's activations
                w_cache = {}

                def load_w(ht):
                    # both wg halves before wu: the opening matmuls of
                    # each h-tile consume wg only
                    wg_sb, wu_sb = [], []
                    for hh in range(2):
                        w1 = wp.tile([P, HH, P], dt_in, tag=f"wg{hh}",
                                     name=f"wg{hh}")
                        nc.sync.dma_start(
                            w1[:], wgt[ht, :, hh * HH : (hh + 1) * HH])
                        wg_sb.append(w1)
                    for hh in range(2):
                        w2 = wp.tile([P, HH, P], dt_in, tag=f"wu{hh}",
                                     name=f"wu{hh}")
                        nc.sync.dma_start(
                            w2[:], wut[ht, :, hh * HH : (hh + 1) * HH])
                        wu_sb.append(w2)
                    w_cache[ht] = (wg_sb, wu_sb)

                load_w(0)

                # first k-slice of x as its own tile so the opening
                # matmuls don't wait for the whole chunk's activations
                x_sb = []
                for gi, (off, csize) in enumerate(group):
                    x0 = xp.tile([P, csize], dt_in, tag=f"x{gi}k0",
                                 bufs=2 if gi == 0 else 1, name=f"x{gi}k0")
                    nc.sync.dma_start(x0[:], xt[:, 0, off : off + csize])
                    xr = xp.tile([P, KD - 1, csize], dt_in, tag=f"x{gi}r",
                                 bufs=2 if gi == 0 else 1, name=f"x{gi}r")
                    nc.sync.dma_start(xr[:], xt[:, 1:, off : off + csize])
                    x_sb.append([x0] + [xr[:, kt] for kt in range(KD - 1)])

                for ht in range(HT):
                    if ht not in w_cache:
                        load_w(ht)
                    wg_sb, wu_sb = w_cache.pop(ht)

                    for gi, (off, csize) in enumerate(group):
                        pg = ps.tile([P, csize], f32, tag="pg", bufs=3)
                        pu = ps.tile([P, csize], f32, tag="pu", bufs=3)
                        for kt in range(KD):
                            nc.tensor.matmul(
                                pg,
                                wg_sb[kt // HH][:, kt % HH],
                                x_sb[gi][kt],
                                start=(kt == 0),
                                stop=(kt == KD - 1),
                            )
                        for kt in range(KD):
                            nc.tensor.matmul(
                                pu,
                                wu_sb[kt // HH][:, kt % HH],
                                x_sb[gi][kt],
                                start=(kt == 0),
                                stop=(kt == KD - 1),
                            )
                        sl = op.tile([P, csize], f32, tag="silu")
                        nc.scalar.activation(sl[:], pg, SiLU)
                        lo = off - g_off
                        nc.vector.tensor_mul(
                            h_sb[:, ht, lo : lo + csize], sl[:], pu
                        )

                # y = h @ Wd, feature-major [DIM, g_size]
                for dt in range(DT):
                    wd_sb = wp.tile([P, HT, P], dt_in, tag="wd")
                    nc.sync.dma_start(wd_sb[:], wdt[dt])
                    for gi, (off, csize) in enumerate(group):
                        py = ps.tile([P, csize], f32, tag="py")
                        lo = off - g_off
                        for ht in range(HT):
                            nc.tensor.matmul(
                                py,
                                wd_sb[:, ht],
                                h_sb[:, ht, lo : lo + csize],
                                start=(ht == 0),
                                stop=(ht == HT - 1),
                            )
                        o_sb = op.tile([P, csize], f32, tag="o")
                        nc.vector.tensor_copy(o_sb[:], py)
                        nc.sync.dma_start(yt[dt, :, off : off + csize], o_sb[:])

    nc.finalize()
    return nc


def _get_kernel(C):
    if C not in _KERNEL_CACHE:
        _KERNEL_CACHE[C] = _build_moe_ffn(C)
    return _KERNEL_CACHE[C]


def _np_dtype():
    if DTYPE == "bf16":
        import ml_dtypes

        return np.dtype(ml_dtypes.bfloat16)
    return np.dtype(np.float32)


def _route(xf, W_gate):
    """Replicate reference routing: top-2 by logit, softmax weights.

    float64 logits: the top-k decision boundary gap is >> f32 rounding
    noise, so this matches the f32 jax reference's selection."""
    logits = xf.astype(np.float64) @ W_gate.astype(np.float64)  # [N, E]
    order = np.argsort(-logits, axis=1, kind="stable")[:, :TOPK]  # [N, 2]
    top = np.take_along_axis(logits, order, axis=1)
    top = top - top.max(axis=1, keepdims=True)
    ew = np.exp(top)
    w = (ew / ew.sum(axis=1, keepdims=True)).astype(np.float32)  # [N, 2]
    return order, w


def kernel(x, W_gate, Wg, Wu, Wd):
    from concourse.bass_utils import run_bass_kernel_spmd

    x = np.ascontiguousarray(np.asarray(x, dtype=np.float32))
    W_gate = np.asarray(W_gate, dtype=np.float32)
    Wg = np.asarray(Wg, dtype=np.float32)
    Wu = np.asarray(Wu, dtype=np.float32)
    Wd = np.asarray(Wd, dtype=np.float32)

    B, T, D = x.shape
    xf = x.reshape(-1, D)
    N = xf.shape[0]

    order, w = _route(xf, W_gate)

    ids = []  # per-expert token indices
    wts = []  # per-expert combine weights
    for e in range(E):
        sel = np.nonzero(order == e)
        ids.append(sel[0])
        wts.append(w[sel[0], sel[1]])

    max_cnt = max(len(i) for i in ids)
    C = _capacity(max_cnt)

    nc = _get_kernel(C)
    ndt = _np_dtype()

    in_maps = []
    for e in range(E):
        cnt = len(ids[e])
        xe = np.zeros((C, DIM), dtype=np.float32)
        xe[:cnt] = xf[ids[e]]
        x_t = np.ascontiguousarray(
            xe.T.reshape(KD, P, C).transpose(1, 0, 2).astype(ndt, copy=False)
        )
        wg_t = np.ascontiguousarray(
            Wg[e].reshape(KD, P, HT, P).transpose(2, 1, 0, 3).astype(ndt, copy=False)
        )
        wu_t = np.ascontiguousarray(
            Wu[e].reshape(KD, P, HT, P).transpose(2, 1, 0, 3).astype(ndt, copy=False)
        )
        wd_t = np.ascontiguousarray(
            Wd[e].reshape(HT, P, DT, P).transpose(2, 1, 0, 3).astype(ndt, copy=False)
        )
        in_maps.append({"xt": x_t, "wgt": wg_t, "wut": wu_t, "wdt": wd_t})

    res = run_bass_kernel_spmd(nc, in_maps, core_ids=list(range(E)))
    global LAST_RESULTS
    LAST_RESULTS = res

    out = np.zeros((N, D), dtype=np.float32)
    for e in range(E):
        cnt = len(ids[e])
        y_e = res.results[e]["yt"].reshape(DIM, C)[:, :cnt].T  # [cnt, D]
        out[ids[e]] += wts[e][:, None] * y_e
    return out.reshape(B, T, D)



# revision 1
# speedup vs baseline: 1.1330x; 1.1330x over previous
"""Expert-parallel MoE (top-2 of 8 experts, SwiGLU FFN) for 8 Trainium2 cores.

Strategy (matches the expert-parallel sharding hint):
  - Host computes the small gate (logits -> top-2 -> softmax) in float64
    numpy, then dispatches ("all-to-all" on host) tokens to experts.
  - Core e holds expert e's weights and runs a dense SwiGLU FFN over the
    tokens routed to expert e (padded to a common capacity C so all 8
    cores run the same SPMD program).
  - The device kernel works entirely in "feature-major" layout (features
    on partitions, tokens on the free axis) so the h = silu(x@Wg)*(x@Wu)
    intermediate feeds the down-projection without any transpose.
  - Host applies the routing weights and scatter-adds the per-expert
    outputs back into the full [B,T,D] output.

DTYPE selects the matmul path (both measured on HW):
  - "f32r" (default): fp32 bits on the relaxed-precision PE path,
    1 cycle/row + ~40 cycle/matmul weight-load bubble. Max-core HW time
    ~561 us, rel err vs f64 reference 2.6e-4.
  - "bf16": inputs rounded to bfloat16, 1 cycle/row, LDWEIGHTS hidden by
    fast-weight-load. ~508 us but rel err 4.1e-3 — kept as a fallback.
"""

import numpy as np

DIM = 1024
HID = 2816
E = 8
TOPK = 2
P = 128
KD = DIM // P   # 8 k-subtiles (contraction of x@W)
HT = HID // P   # 22 h-subtiles
DT = DIM // P   # 8 d-subtiles (output features)

DTYPE = "f32r"          # "bf16" | "f32r"
CHUNK_MAX = 512         # PSUM bank limit (512 fp32 accumulators)
CHUNK_MIN = 384         # keep weight-DMA per chunk sustainable

_KERNEL_CACHE = {}
LAST_RESULTS = None  # BassKernelResults of the most recent run (for profiling)


def _align():
    # f32r matmuls reject odd moving free dims (walrus ISA check);
    # bf16 takes any size.
    return 1 if DTYPE == "bf16" else 2


def _capacity(max_cnt):
    # chunks produced by _build_chunks are all >= CHUNK_MIN >= 256, so
    # both the bf16 and f32r matmul fast paths allow any capacity.
    a = _align()
    return max(CHUNK_MIN, ((max_cnt + a - 1) // a) * a)


def _build_chunks(C):
    """Split C into aligned chunks in [CHUNK_MIN, CHUNK_MAX], largest
    LAST: a trailing single-chunk weight group streams 1MB of wg/wu per
    h-tile, and only a full-width chunk keeps that under the ~360GB/s
    per-core HBM roofline."""
    count = (C + CHUNK_MAX - 1) // CHUNK_MAX
    sizes = []
    rem = C
    for i in range(count, 0, -1):
        if i == 1:
            s = rem
        else:
            s = min(CHUNK_MAX, rem - CHUNK_MIN * (i - 1))
        sizes.append(s)
        rem -= s
    sizes.reverse()
    assert all(CHUNK_MIN <= s <= CHUNK_MAX and s % _align() == 0 for s in sizes)
    chunks = []
    off = 0
    for s in sizes:
        chunks.append((off, s))
        off += s
    return chunks


def _build_groups(chunks, group_size):
    return [chunks[i : i + group_size] for i in range(0, len(chunks), group_size)]


def _build_moe_ffn(C):
    """Build the per-core Bass program: y^T = SwiGLU FFN of x^T, both
    feature-major, tokens padded to capacity C."""
    import concourse.bass as bass  # noqa: F401
    import concourse.mybir as mybir
    from concourse import bacc, tile

    f32 = mybir.dt.float32
    dt_in = mybir.dt.bfloat16 if DTYPE == "bf16" else mybir.dt.float32r
    SiLU = mybir.ActivationFunctionType.Silu

    nc = bacc.Bacc("TRN2", target_bir_lowering=False, debug=False)

    xt = nc.dram_tensor("xt", [P, KD, C], dt_in, kind="ExternalInput")
    wgt = nc.dram_tensor("wgt", [HT, P, KD, P], dt_in, kind="ExternalInput")
    wut = nc.dram_tensor("wut", [HT, P, KD, P], dt_in, kind="ExternalInput")
    wdt = nc.dram_tensor("wdt", [DT, P, HT, P], dt_in, kind="ExternalInput")
    yt = nc.dram_tensor("yt", [DT, P, C], f32, kind="ExternalOutput")

    # bf16 halves weight DMA, so single-chunk groups sustain; f32r needs
    # two chunks per weight pass to stay under the HBM roofline.
    group_size = 1 if DTYPE == "bf16" else 2
    groups = _build_groups(_build_chunks(C), group_size)

    with tile.TileContext(nc) as tc:
        with (
            tc.tile_pool(name="xp", bufs=1) as xp,
            tc.tile_pool(name="wp", bufs=3) as wp,
            tc.tile_pool(name="hp", bufs=2 if group_size == 1 else 1) as hp,
            tc.tile_pool(name="op", bufs=3) as op,
            tc.tile_pool(name="ps", bufs=2, space="PSUM") as ps,
        ):
            HH = KD // 2  # wg/wu tiles split in halves for earlier start
            for group in groups:
                g_off = group[0][0]
                g_size = sum(c[1] for c in group)

                # h = silu(x @ Wg) * (x @ Wu), feature-major [HID, g_size]
                h_sb = hp.tile([P, HT, g_size], dt_in, tag="h")

                # ht=0 weights first so the opening matmuls wait on
                # ~0.75MB, not the whole group's activations
                w_cache = {}

                def load_w(ht):
                    # both wg halves before wu: the opening matmuls of
                    # each h-tile consume wg only
                    wg_sb, wu_sb = [], []
                    for hh in range(2):
                        w1 = wp.tile([P, HH, P], dt_in, tag=f"wg{hh}",
                                     name=f"wg{hh}")
                        nc.sync.dma_start(
                            w1[:], wgt[ht, :, hh * HH : (hh + 1) * HH])
                        wg_sb.append(w1)
                    for hh in range(2):
                        w2 = wp.tile([P, HH, P], dt_in, tag=f"wu{hh}",
                                     name=f"wu{hh}")
                        nc.sync.dma_start(
                            w2[:], wut[ht, :, hh * HH : (hh + 1) * HH])
                        wu_sb.append(w2)
                    w_cache[ht] = (wg_sb, wu_sb)

                load_w(0)

                # first k-slice of x as its own tile so the opening
                # matmuls don't wait for the whole chunk's activations
                x_sb = []
                for gi, (off, csize) in enumerate(group):
                    x0 = xp.tile([P, csize], dt_in, tag=f"x{gi}k0",
                                 bufs=2 if gi == 0 else 1, name=f"x{gi}k0")
                    nc.sync.dma_start(x0[:], xt[:, 0, off : off + csize])
                    xr = xp.tile([P, KD - 1, csize], dt_in, tag=f"x{gi}r",
                                 bufs=2 if gi == 0 else 1, name=f"x{gi}r")
                    nc.sync.dma_start(xr[:], xt[:, 1:, off : off + csize])
                    x_sb.append([x0] + [xr[:, kt] for kt in range(KD - 1)])

                for ht in range(HT):
                    if ht not in w_cache:
                        load_w(ht)
                    wg_sb, wu_sb = w_cache.pop(ht)

                    for gi, (off, csize) in enumerate(group):
                        pg = ps.tile([P, csize], f32, tag="pg", bufs=3)
                        pu = ps.tile([P, csize], f32, tag="pu", bufs=3)
                        for kt in range(KD):
                            nc.tensor.matmul(
                                pg,
                                wg_sb[kt // HH][:, kt % HH],
                                x_sb[gi][kt],
                                start=(kt == 0),
                                stop=(kt == KD - 1),
                            )
                        for kt in range(KD):
                            nc.tensor.matmul(
                                pu,
                                wu_sb[kt // HH][:, kt % HH],
                                x_sb[gi][kt],
                                start=(kt == 0),
                                stop=(kt == KD - 1),
                            )
                        sl = op.tile([P, csize], f32, tag="silu")
                        nc.scalar.activation(sl[:], pg, SiLU)
                        lo = off - g_off
                        nc.vector.tensor_mul(
                            h_sb[:, ht, lo : lo + csize], sl[:], pu
                        )

                # y = h @ Wd, feature-major [DIM, g_size]
                for dt in range(DT):
                    wd_sb = wp.tile([P, HT, P], dt_in, tag="wd")
                    nc.sync.dma_start(wd_sb[:], wdt[dt])
                    for gi, (off, csize) in enumerate(group):
                        py = ps.tile([P, csize], f32, tag="py")
                        lo = off - g_off
                        for ht in range(HT):
                            nc.tensor.matmul(
                                py,
                                wd_sb[:, ht],
                                h_sb[:, ht, lo : lo + csize],
                                start=(ht == 0),
                                stop=(ht == HT - 1),
                            )
                        o_sb = op.tile([P, csize], f32, tag="o")
                        nc.vector.tensor_copy(o_sb[:], py)
                        nc.sync.dma_start(yt[dt, :, off : off + csize], o_sb[:])

    nc.finalize()
    return nc


def _get_kernel(C):
    if C not in _KERNEL_CACHE:
        _KERNEL_CACHE[C] = _build_moe_ffn(C)
    return _KERNEL_CACHE[C]


def _np_dtype():
    if DTYPE == "bf16":
        import ml_dtypes

        return np.dtype(ml_dtypes.bfloat16)
    return np.dtype(np.float32)


def _route(xf, W_gate):
    """Replicate reference routing: top-2 by logit, softmax weights.

    float64 logits: the top-k decision boundary gap is >> f32 rounding
    noise, so this matches the f32 jax reference's selection."""
    logits = xf.astype(np.float64) @ W_gate.astype(np.float64)  # [N, E]
    order = np.argsort(-logits, axis=1, kind="stable")[:, :TOPK]  # [N, 2]
    top = np.take_along_axis(logits, order, axis=1)
    top = top - top.max(axis=1, keepdims=True)
    ew = np.exp(top)
    w = (ew / ew.sum(axis=1, keepdims=True)).astype(np.float32)  # [N, 2]
    return order, w


def kernel(x, W_gate, Wg, Wu, Wd):
    from concourse.bass_utils import run_bass_kernel_spmd

    x = np.ascontiguousarray(np.asarray(x, dtype=np.float32))
    W_gate = np.asarray(W_gate, dtype=np.float32)
    Wg = np.asarray(Wg, dtype=np.float32)
    Wu = np.asarray(Wu, dtype=np.float32)
    Wd = np.asarray(Wd, dtype=np.float32)

    B, T, D = x.shape
    xf = x.reshape(-1, D)
    N = xf.shape[0]

    order, w = _route(xf, W_gate)

    ids = []  # per-expert token indices
    wts = []  # per-expert combine weights
    for e in range(E):
        sel = np.nonzero(order == e)
        ids.append(sel[0])
        wts.append(w[sel[0], sel[1]])

    max_cnt = max(len(i) for i in ids)
    C = _capacity(max_cnt)

    nc = _get_kernel(C)
    ndt = _np_dtype()

    in_maps = []
    for e in range(E):
        cnt = len(ids[e])
        xe = np.zeros((C, DIM), dtype=np.float32)
        xe[:cnt] = xf[ids[e]]
        x_t = np.ascontiguousarray(
            xe.T.reshape(KD, P, C).transpose(1, 0, 2).astype(ndt, copy=False)
        )
        wg_t = np.ascontiguousarray(
            Wg[e].reshape(KD, P, HT, P).transpose(2, 1, 0, 3).astype(ndt, copy=False)
        )
        wu_t = np.ascontiguousarray(
            Wu[e].reshape(KD, P, HT, P).transpose(2, 1, 0, 3).astype(ndt, copy=False)
        )
        wd_t = np.ascontiguousarray(
            Wd[e].reshape(HT, P, DT, P).transpose(2, 1, 0, 3).astype(ndt, copy=False)
        )
        in_maps.append({"xt": x_t, "wgt": wg_t, "wut": wu_t, "wdt": wd_t})

    res = run_bass_kernel_spmd(nc, in_maps, core_ids=list(range(E)))
    global LAST_RESULTS
    LAST_RESULTS = res

    out = np.zeros((N, D), dtype=np.float32)
    for e in range(E):
        cnt = len(ids[e])
        y_e = res.results[e]["yt"].reshape(DIM, C)[:, :cnt].T  # [cnt, D]
        out[ids[e]] += wts[e][:, None] * y_e
    return out.reshape(B, T, D)

